# revision 4
# baseline (speedup 1.0000x reference)
"""Correlation network kernel for Trainium2.

corr[b,i,j,k,l] = sum_c A[b,i,j,c] * B[b,k,l,c]

Per batch b this is  A_b (2304x64) @ B_b^T (64x2304) -> 2304x2304.
Sharding: data-parallel over batch B=8 across the 8 NeuronCores; each core
computes one full 2304x2304 correlation matrix, emitted as BF16 (10.6
MB/core; host upcasts). The joint floor is the PE column rate (41472
output columns at ~0.73 ns/col ~= 30.4 us; the PE drains exactly one
128-row output column per cycle regardless of dtype/perf-mode) and the
~358 GB/s HBM write wire (~29.7 us), plus the ~7.3 us Tile preamble.

v4 design:
  - fp8 e4m3 DoubleRow matmuls: A = A_hi + A_lo, B = B_hi + B_lo (fp8
    hi/lo splits). K=256 packed 2-per-partition: partition p holds
    channel p%64 of A_hi (p<64) / A_lo (p>=64); k-tile 0 pairs with
    B_hi, k-tile 1 with B_lo. One DoubleRow matmul per (m-tile, n-bank)
    computes the full (A_hi+A_lo)@(B_hi+B_lo) product; fro err ~2e-3.
    (Same column rate as bf16, but a single matmul per bank instead of
    two keeps the PE queue short; PE measures ~100% busy.)
  - Wide PSUM->SBUF casts: [128,1024] 2-bank copies alternating DVE/ACT
    (~1.4 us/engine/m-tile < the ~1.7 us/m-tile write pace).
    PSUM = 3x 2-bank + 2x 1-bank pools = exactly 8 banks.
  - Output staging: ONE persistent [128, 41472] bf16 SBUF tile (81
    KB/partition) holding the whole result in m-tile-major column
    blocks; out_dram[r, m*2304+c] = corr[m*128+r, c] (host unpacks).
    Flushed as 20 DMAs of exactly 4096 B/partition + one 1 KB tail on
    the sync HWDGE ring - clean power-of-two descriptors keep the wire
    at full rate (4608 B patterns measured ~270 GB/s aggregate; 4 KB
    ~358), issued incrementally as column blocks complete so the write
    stream trails the PE by <1 us.
"""

import numpy as np
import ml_dtypes

import concourse.bacc as bacc
import concourse.mybir as mybir
import concourse.tile as tile
from concourse.bass_interp import get_hw_module
from concourse.bass_utils import run_bass_kernel_spmd

B, H, W, C = 8, 48, 48, 64
HW = H * W  # 2304
P = 128
M_TILES = HW // P  # 18
FP32 = mybir.dt.float32
BF16 = mybir.dt.bfloat16
FP8 = mybir.dt.float8e4
BF16_NP = ml_dtypes.bfloat16
FP8_NP = ml_dtypes.float8_e4m3
DR = mybir.MatmulPerfMode.DoubleRow
TOT = M_TILES * HW  # 41472 output columns in the staging tile
CHUNK = 2048  # bf16 cols per DMA = 4096 B/partition


def _corr_body(tc, out, lhs_h, rhs_h):
    nc = tc.nc
    with (
        tc.tile_pool(name="ops", bufs=1) as op_pool,
        tc.tile_pool(name="psw", bufs=3, space="PSUM") as ps_wide,
        tc.tile_pool(name="pst", bufs=2, space="PSUM") as ps_tail,
    ):
        lt = op_pool.tile([P, 2, HW], FP8)
        rt = op_pool.tile([P, 2, HW], FP8)
        ot = op_pool.tile([P, TOT], BF16)
        # Input loads: first chunks sized so m-tile 0's matmuls never
        # stall; rest streams behind on both HWDGE rings.
        nc.sync.dma_start(out=rt[:, :, 0:1024], in_=rhs_h[:, :, 0:1024])
        nc.sync.dma_start(out=rt[:, :, 1024:HW], in_=rhs_h[:, :, 1024:HW])
        nc.scalar.dma_start(out=lt[:, :, 0:256], in_=lhs_h[:, :, 0:256])
        nc.scalar.dma_start(out=lt[:, :, 256:HW], in_=lhs_h[:, :, 256:HW])

        flushed = 0
        for m in range(M_TILES):
            base = m * HW
            lcol = lt[:, :, m * P : (m + 1) * P]
            t0 = ps_wide.tile([P, 1024], FP32, tag="ps")
            t1 = ps_wide.tile([P, 1024], FP32, tag="ps")
            t2 = ps_tail.tile([P, 512], FP32, tag="pt")
            for ps, o0, o1 in (
                (t0[:, 0:512], 0, 512),
                (t0[:, 512:1024], 512, 1024),
                (t1[:, 0:512], 1024, 1536),
                (t1[:, 512:1024], 1536, 2048),
                (t2[:, 0:256], 2048, 2304),
            ):
                nc.tensor.matmul(
                    ps, lcol, rt[:, :, o0:o1], start=True, stop=True, perf_mode=DR
                )
            # 3 copies per m-tile; alternate which engine takes the pair
            # {1024} vs {1024, 256} so DVE/ACT stay balanced. The last
            # m-tile puts the small tail copy last on its own engine so
            # the post-PE copy tail is short.
            if m == M_TILES - 1:
                nc.vector.tensor_copy(ot[:, base : base + 1024], t0[:, :])
                nc.scalar.copy(ot[:, base + 1024 : base + 2048], t1[:, :])
                nc.vector.tensor_copy(ot[:, base + 2048 : base + 2304], t2[:, 0:256])
            elif m % 2 == 0:
                nc.vector.tensor_copy(ot[:, base : base + 1024], t0[:, :])
                nc.scalar.copy(ot[:, base + 1024 : base + 2048], t1[:, :])
                nc.scalar.copy(ot[:, base + 2048 : base + 2304], t2[:, 0:256])
            else:
                nc.scalar.copy(ot[:, base : base + 1024], t0[:, :])
                nc.vector.tensor_copy(ot[:, base + 1024 : base + 2048], t1[:, :])
                nc.vector.tensor_copy(ot[:, base + 2048 : base + 2304], t2[:, 0:256])
            # flush every completed 4 KB/partition column block
            avail = base + HW
            while flushed + CHUNK <= avail:
                nc.sync.dma_start(
                    out=out[:, flushed : flushed + CHUNK],
                    in_=ot[:, flushed : flushed + CHUNK],
                )
                flushed += CHUNK
        nc.sync.dma_start(out=out[:, flushed:TOT], in_=ot[:, flushed:TOT])


_NC_CACHE = None


def _build():
    global _NC_CACHE
    if _NC_CACHE is None:
        nc = bacc.Bacc(
            "TRN2",
            target_bir_lowering=False,
            debug=False,
            enable_asserts=False,
        )
        lhs_h = nc.dram_tensor("lhs_h", [P, 2, HW], FP8, kind="ExternalInput").ap()
        rhs_h = nc.dram_tensor("rhs_h", [P, 2, HW], FP8, kind="ExternalInput").ap()
        out = nc.dram_tensor("out", [P, TOT], BF16, kind="ExternalOutput").ap()
        with tile.TileContext(nc) as tc:
            _corr_body(tc, out, lhs_h, rhs_h)
        nc.compile()
        nc.m = get_hw_module(nc.m)
        _NC_CACHE = nc
    return _NC_CACHE


def _prep_inputs(feature_A, feature_B):
    in_maps = []
    for i in range(B):
        A2 = np.ascontiguousarray(feature_A[i].reshape(HW, C), dtype=np.float32)
        B2 = np.ascontiguousarray(feature_B[i].reshape(HW, C), dtype=np.float32)
        ah = A2.astype(FP8_NP)
        al = (A2 - ah.astype(np.float32)).astype(FP8_NP)
        bh = B2.astype(FP8_NP)
        bl = (B2 - bh.astype(np.float32)).astype(FP8_NP)
        # lhs [128, 2, 2304]: partition p<64 = A_hi ch p, p>=64 = A_lo
        # ch p-64; identical across the two k-tiles.
        apart = np.concatenate([ah.T, al.T], axis=0)  # [128, 2304]
        lhs = np.stack([apart, apart], axis=1)  # [128, 2, 2304]
        # rhs [128, 2, 2304]: k-tile 0 = B_hi ch p%64, k-tile 1 = B_lo.
        r0 = np.concatenate([bh.T, bh.T], axis=0)
        r1 = np.concatenate([bl.T, bl.T], axis=0)
        rhs = np.stack([r0, r1], axis=1)
        in_maps.append(
            {
                "lhs_h": np.ascontiguousarray(lhs),
                "rhs_h": np.ascontiguousarray(rhs),
            }
        )
    return in_maps


def _unpack_out(o):
    """[128, 41472] m-tile-major -> [2304, 2304] fp32."""
    o = np.asarray(o).reshape(P, M_TILES, HW)
    return o.transpose(1, 0, 2).reshape(HW, HW).astype(np.float32)


def _run(feature_A, feature_B, trace=False, **kwargs):
    feature_A = np.asarray(feature_A, dtype=np.float32)
    feature_B = np.asarray(feature_B, dtype=np.float32)
    assert feature_A.shape == (B, H, W, C), feature_A.shape
    assert feature_B.shape == (B, H, W, C), feature_B.shape

    nc = _build()
    in_maps = _prep_inputs(feature_A, feature_B)
    res = run_bass_kernel_spmd(nc, in_maps, list(range(B)), trace=trace, **kwargs)
    out = np.stack([_unpack_out(res.results[i]["out"]) for i in range(B)], axis=0)
    return out.reshape(B, H, W, H, W), res


def kernel(feature_A, feature_B):
    out, _ = _run(feature_A, feature_B)
    return out
